# revision 1
# baseline (speedup 1.0000x reference)
"""Trainium2 Bass kernel for nn_AutodiffChannel: 6-biquad EQ cascade over
(64, 1, 262144) fp32 audio, data-parallel over 8 NeuronCores.

Algorithm (per sequence, LTI block-state decomposition):
  The 6-stage DF2T biquad cascade is a 12-state linear system
  s' = A s + B x, y = C s + D x.  Split T=262144 into 2048 chunks of
  L=128.  Then per chunk c:
      y_c = Phi x_c + Gamma S_c          (Phi  = 128x128 lower-tri Toeplitz
                                          of the impulse response h[0:128],
                                          Gamma[m,:] = C A^m)
      U_c = M x_c                        (M[:,n] = A^(127-n) B)
      S_c = sum_{j<c} (A^128)^(c-1-j) U_j   (exclusive prefix "state scan")
  The prefix is computed with a Kogge-Stone scan (11 levels) using
  precomputed powers P_d = (A^128)^(2^d).  All the O(T) work (Phi/M/Gamma
  matmuls + scan) runs on the PE/DVE/ACT engines; the tiny per-sequence
  setup (h, Gamma, M, P_d: ~0.1% of total FLOPs, independent of T) is
  computed host-side in float64 from the fp32-quantized biquad coeffs.

Device dataflow per core (8 sequences):
  x arrives pre-split into bf16 hi/lo pairs (host does the exact split);
  PE transposes both into chunk-column layout XTh/XTl (column q = j*128+p
  holds chunk c = p*16+j).  U = M x and the FIR/correction matmuls use
  3-term bf16 split-precision (Wh*xh + Wh*xl + Wl*xh, fp32 PSUM accum,
  ~2^-17 relative error) so the PE runs at full 1 col/cycle instead of
  fp32's 4 cycles/col.  The 11-level Kogge-Stone state scan stays fp32 on
  a 96-row (8 seq x 12 states) buffer.  Phase B emits y in chunk-column
  layout, PE-transposes back to natural fp32, and DMAs out.
"""
import sys

for _p in ("/opt/trn_rl_repo", "/opt/trn_rl_repo/concourse"):
    if _p not in sys.path:
        sys.path.insert(0, _p)

import numpy as np

import concourse.bacc as bacc
import concourse.mybir as mybir
from concourse.tile import TileContext
from concourse.bass_utils import run_bass_kernel_spmd

# ---------------------------------------------------------------- problem dims
B, C, T = 64, 1, 262144
N_CORES = 8
SEQ_PER_CORE = B * C // N_CORES  # 8
L = 128                     # chunk length
NCH = T // L                # 2048 chunks per sequence
ROWS = 128                  # natural-layout partitions per sequence
COLS = T // ROWS            # 2048
JG = COLS // L              # 16 chunk-interleave factor (c = p*16 + j)
LEVELS = 11                 # ceil(log2(NCH))
NSTATE = 12
F32 = mybir.dt.float32
F32R = mybir.dt.float32r
BF16 = mybir.dt.bfloat16

PARAM_RANGES = np.array([
    [-24.0, 24.0], [20.0, 200.0], [0.1, 10.0],
    [-24.0, 24.0], [200.0, 2000.0], [0.1, 10.0],
    [-24.0, 24.0], [200.0, 2000.0], [0.1, 10.0],
    [-24.0, 24.0], [2000.0, 8000.0], [0.1, 10.0],
    [-24.0, 24.0], [4000.0, 12000.0], [0.1, 10.0],
    [-24.0, 24.0], [4000.0, 12000.0], [0.1, 10.0],
], dtype=np.float32)
FILTER_TYPES = ["low_shelf", "peaking", "peaking", "peaking", "peaking",
                "high_shelf"]


# ------------------------------------------------------------- host-side setup
def _sigmoid_f32(z):
    z = z.astype(np.float32)
    out = np.empty_like(z)
    pos = z >= 0
    out[pos] = (np.float32(1.0) / (np.float32(1.0) + np.exp(-z[pos]))).astype(
        np.float32)
    ez = np.exp(z[~pos]).astype(np.float32)
    out[~pos] = (ez / (np.float32(1.0) + ez)).astype(np.float32)
    return out


def _biquad_coeffs_f32(g, f, q, sr, ftype):
    """fp32-faithful audio-EQ-cookbook coefficients (matches reference)."""
    f32 = np.float32
    A = np.power(f32(10.0), (g / f32(40.0)).astype(f32)).astype(f32)
    w0 = (f32(2.0) * f32(np.pi) * (f / f32(sr))).astype(f32)
    alpha = (np.sin(w0, dtype=f32) / (f32(2.0) * q)).astype(f32)
    c = np.cos(w0, dtype=f32)
    sA = np.sqrt(A).astype(f32)
    one, two = f32(1.0), f32(2.0)
    if ftype == "low_shelf":
        b0 = A * ((A + one) - (A - one) * c + two * sA * alpha)
        b1 = two * A * ((A - one) - (A + one) * c)
        b2 = A * ((A + one) - (A - one) * c - two * sA * alpha)
        a0 = (A + one) + (A - one) * c + two * sA * alpha
        a1 = -two * ((A - one) + (A + one) * c)
        a2 = (A + one) + (A - one) * c - two * sA * alpha
    elif ftype == "high_shelf":
        b0 = A * ((A + one) + (A - one) * c + two * sA * alpha)
        b1 = -two * A * ((A - one) + (A + one) * c)
        b2 = A * ((A + one) + (A - one) * c - two * sA * alpha)
        a0 = (A + one) - (A - one) * c + two * sA * alpha
        a1 = two * ((A - one) - (A + one) * c)
        a2 = (A + one) - (A - one) * c - two * sA * alpha
    else:
        b0 = one + alpha * A
        b1 = -two * c
        b2 = one - alpha * A
        a0 = one + alpha / A
        a1 = -two * c
        a2 = one - alpha / A
    bc = (np.stack([b0, b1, b2], -1).astype(f32) / a0[..., None]).astype(f32)
    ac = (np.stack([a0, a1, a2], -1).astype(f32) / a0[..., None]).astype(f32)
    return bc, ac


def _coeffs_from_inputs(p, W, b, sample_rate):
    z = (p.astype(np.float32) @ W.astype(np.float32).T
         + b.astype(np.float32)).astype(np.float32)
    pn = _sigmoid_f32(z)
    lo, hi = PARAM_RANGES[:, 0], PARAM_RANGES[:, 1]
    params = (pn * (hi - lo) + lo).astype(np.float32)
    bcs, acs = [], []
    for k, ftype in enumerate(FILTER_TYPES):
        bc, ac = _biquad_coeffs_f32(
            params[:, 3 * k], params[:, 3 * k + 1], params[:, 3 * k + 2],
            float(sample_rate), ftype)
        bcs.append(bc)
        acs.append(ac)
    return np.stack(bcs), np.stack(acs)  # (6, B, 3) fp32


def _state_space(bc, ac):
    """Vectorized float64 (A, B, C, D) per sequence from fp32 DF2T coeffs."""
    nb = bc.shape[1]
    bc64 = bc.astype(np.float64)
    ac64 = ac.astype(np.float64)

    def step(s, x):
        # s: (nb, 12); x: (nb,) -> s', y
        s = s.copy()
        v = x
        for k in range(6):
            b0, b1, b2 = bc64[k, :, 0], bc64[k, :, 1], bc64[k, :, 2]
            a1, a2 = ac64[k, :, 1], ac64[k, :, 2]
            s1, s2 = s[:, 2 * k], s[:, 2 * k + 1]
            y = b0 * v + s1
            s[:, 2 * k] = b1 * v - a1 * y + s2
            s[:, 2 * k + 1] = b2 * v - a2 * y
            v = y
        return s, v

    A = np.zeros((nb, NSTATE, NSTATE))
    Cv = np.zeros((nb, NSTATE))
    for i in range(NSTATE):
        e = np.zeros((nb, NSTATE))
        e[:, i] = 1.0
        sp, y = step(e, np.zeros(nb))
        A[:, :, i] = sp
        Cv[:, i] = y
    Bv, D = step(np.zeros((nb, NSTATE)), np.ones(nb))
    return A, Bv, Cv, D


def _derived(A, Bv, Cv, D):
    """h (nb,L), Gamma (nb,L,12), M (nb,12,L), Pd (nb,LEVELS,12,12) in f64."""
    nb = A.shape[0]
    h = np.zeros((nb, L))
    Gam = np.zeros((nb, L, NSTATE))
    M = np.zeros((nb, NSTATE, L))
    h[:, 0] = D
    cam = Cv.copy()          # C A^m
    amb = Bv.copy()          # A^m B
    for m in range(L):
        Gam[:, m, :] = cam
        M[:, :, L - 1 - m] = amb
        if m + 1 < L:
            h[:, m + 1] = np.einsum("bi,bi->b", cam, Bv)
        cam = np.einsum("bi,bij->bj", cam, A)
        amb = np.einsum("bij,bj->bi", A, amb)
    sq = A.copy()
    for _ in range(7):       # A^(2^7) = A^128
        sq = sq @ sq
    Pd = np.zeros((nb, LEVELS, NSTATE, NSTATE))
    for d in range(LEVELS):
        Pd[:, d] = sq
        sq = sq @ sq
    return h, Gam, M, Pd


def _split_hi_lo(a):
    """Split fp32 into bf16 hi + bf16 lo (a ~= hi + lo, ~17-bit mantissa)."""
    import ml_dtypes
    a = a.astype(np.float32)
    hi = a.astype(ml_dtypes.bfloat16)
    lo = (a - hi.astype(np.float32)).astype(ml_dtypes.bfloat16)
    return hi, lo


def _pack_weights(h, Gam, M, Pd):
    """fp32 device weight tensors, per core."""
    nb = h.shape[0]
    m_idx = np.arange(L)
    diff = m_idx[None, :] - m_idx[:, None]          # [n, m] = m - n
    toepT = np.where(diff >= 0, h[:, np.clip(diff, 0, L - 1)],
                     0.0).astype(np.float32)        # (nb, n=128, m=128)
    # embedded at per-seq 12-row offsets inside a 96-row frame so every
    # device access stays at base partition 0 (HW requires 32-aligned bases)
    gammaT = np.zeros((nb, 96, L), np.float32)      # (nb, k-embed, m)
    mT = np.zeros((nb, L, 96), np.float32)          # (nb, n, k-embed)
    for g in range(nb):
        s8 = g % SEQ_PER_CORE
        gammaT[g, 12 * s8:12 * s8 + 12, :] = Gam[g].T.astype(np.float32)
        mT[g, :, 12 * s8:12 * s8 + 12] = M[g].T.astype(np.float32)
    scanP = np.zeros((N_CORES, LEVELS, 96, 96), np.float32)
    for core in range(N_CORES):
        for s in range(SEQ_PER_CORE):
            g = core * SEQ_PER_CORE + s
            for d in range(LEVELS):
                scanP[core, d, 12 * s:12 * s + 12, 12 * s:12 * s + 12] = \
                    Pd[g, d].T.astype(np.float32)
    return toepT, gammaT, mT, scanP


# ------------------------------------------------------------ device kernel IR
_NC_CACHE = {}


def build_nc(rep=1, ablate=""):
    key = (rep, ablate)
    if key in _NC_CACHE:
        return _NC_CACHE[key]
    nc = bacc.Bacc("TRN2")
    xh_d = nc.dram_tensor("xh", [SEQ_PER_CORE, ROWS, COLS], BF16,
                          kind="ExternalInput")
    xl_d = nc.dram_tensor("xl", [SEQ_PER_CORE, ROWS, COLS], BF16,
                          kind="ExternalInput")
    toepT_d = nc.dram_tensor("toepT", [2, SEQ_PER_CORE, L, L], BF16,
                             kind="ExternalInput")
    gammaT_d = nc.dram_tensor("gammaT", [2, SEQ_PER_CORE, 96, L], BF16,
                              kind="ExternalInput")
    mT_d = nc.dram_tensor("mT", [2, SEQ_PER_CORE, L, 96], BF16,
                          kind="ExternalInput")
    scanP_d = nc.dram_tensor("scanP", [LEVELS, 96, 96], F32,
                             kind="ExternalInput")
    ident_d = nc.dram_tensor("ident", [128, 128], F32, kind="ExternalInput")
    y_d = nc.dram_tensor("y", [SEQ_PER_CORE, ROWS, COLS], F32,
                         kind="ExternalOutput")

    with TileContext(nc) as tc:
        with tc.tile_pool(name="weights", bufs=1) as wpool:
            toepT_sb = wpool.tile([L, 2 * SEQ_PER_CORE * L], BF16)
            nc.sync.dma_start(
                out=toepT_sb[:].rearrange("p (h s m) -> p h s m", m=L, s=8),
                in_=toepT_d[:].transpose([2, 0, 1, 3]))
            gammaT_sb = wpool.tile([96, 2 * SEQ_PER_CORE * L], BF16)
            nc.sync.dma_start(
                out=gammaT_sb[:].rearrange("k (h s m) -> k h s m", m=L, s=8),
                in_=gammaT_d[:].transpose([2, 0, 1, 3]))
            mT_sb = wpool.tile([L, 2 * SEQ_PER_CORE * 96], BF16)
            nc.sync.dma_start(
                out=mT_sb[:].rearrange("n (h s k) -> n h s k", k=96, s=8),
                in_=mT_d[:].transpose([2, 0, 1, 3]))
            scanP_sb = wpool.tile([96, LEVELS * 96], F32)
            nc.sync.dma_start(
                out=scanP_sb[:].rearrange("j (d k) -> j d k", k=96),
                in_=scanP_d[:].transpose([1, 0, 2]))
            ident_sb = wpool.tile([128, 128], F32)
            nc.sync.dma_start(out=ident_sb, in_=ident_d[:])

            with tc.tile_pool(name="xt", bufs=1) as xtpool, \
                 tc.tile_pool(name="xn", bufs=2) as xnpool, \
                 tc.tile_pool(name="ysb", bufs=3) as ypool:
                for _ in range(rep):
                    _one_pass(nc, tc, xh_d, xl_d, y_d, toepT_sb, gammaT_sb,
                              mT_sb, scanP_sb, ident_sb, xtpool,
                              xnpool, ypool, ablate)
    nc.compile()
    _NC_CACHE[key] = nc
    return nc


def _one_pass(nc, tc, xh_d, xl_d, y_d, toepT_sb, gammaT_sb, mT_sb, scanP_sb,
              ident_sb, xtpool, xnpool, ypool, ablate=""):
    # bf16 hi/lo pairs of the chunk-column x (3-term split-precision matmuls)
    XTh = [xtpool.tile([ROWS, COLS], BF16, tag=f"xth{s}", name=f"xth{s}")
           for s in range(SEQ_PER_CORE)]
    XTl = [xtpool.tile([ROWS, COLS], BF16, tag=f"xtl{s}", name=f"xtl{s}")
           for s in range(SEQ_PER_CORE)]

    def wsl(base, h, sq, width):     # hi/lo weight slice helper
        off = (h * SEQ_PER_CORE + sq) * width
        return base[:, off:off + width]

    with tc.tile_pool(name="wbuf", bufs=1) as wbpool:
        # state buffer: col 0 = zeros, col 1+c = inclusive prefix W_c of
        # chunk c; rows 12s..12s+12 = seq s (all accesses use base 0)
        wb = wbpool.tile([96, NCH + 1], F32, tag="wb")
        nc.gpsimd.memset(wb[:, 0:1], 0.0)
        uview = (wb[0:96, 1:NCH + 1]
                 .rearrange("r (p j) -> r p j", j=JG)
                 .transpose([0, 2, 1]))              # (96, j=16, p=128)

        # ---- phase A: x arrives pre-transposed (chunk columns) from host
        with tc.tile_pool(name="up", bufs=2, space="PSUM") as upsum:
            for sq in range(SEQ_PER_CORE):
                nc.sync.dma_start(out=XTh[sq], in_=xh_d[sq])
                nc.sync.dma_start(out=XTl[sq], in_=xl_d[sq])
            # all 8 seqs accumulate into one 96-row U tile per column block
            for i in range(4):
                up = upsum.tile([96, 512], F32, tag="up")
                isl = slice(i * 512, (i + 1) * 512)
                nmm = 3 * SEQ_PER_CORE
                k = 0
                for sq in range(SEQ_PER_CORE):
                    for (wh, xt) in ((0, XTh[sq]), (0, XTl[sq]),
                                     (1, XTh[sq])):
                        nc.tensor.matmul(
                            up[:], lhsT=wsl(mT_sb, wh, sq, 96),
                            rhs=xt[:, isl],
                            start=(k == 0), stop=(k == nmm - 1))
                        k += 1
                nc.vector.tensor_copy(
                    out=uview[:, 4 * i:4 * i + 4, :],
                    in_=up[:].rearrange("r (a b) -> r a b", b=128))

        # ---- state scan (fp32): W_c += P_d W_{c-2^d} ----
        with tc.tile_pool(name="sp", bufs=2, space="PSUM") as spsum:
            for d in range(0 if ablate == "A" else LEVELS):
                sh = 1 << d
                sp = spsum.tile([96, NCH], F32, tag="sp")
                c0 = sh
                while c0 < NCH:
                    c1 = min((c0 // 512 + 1) * 512, NCH)
                    nc.tensor.matmul(
                        sp[:, c0:c1],
                        lhsT=scanP_sb[:, d * 96:(d + 1) * 96],
                        rhs=wb[:, 1 + c0 - sh:1 + c1 - sh],
                        start=True, stop=True)
                    c0 = c1
                nc.vector.tensor_add(out=wb[:, 1 + sh:NCH + 1],
                                     in0=wb[:, 1 + sh:NCH + 1],
                                     in1=sp[:, sh:NCH])

        # ---- split states into bf16 hi/lo for the correction matmuls ----
        with tc.tile_pool(name="shl", bufs=1) as shlpool:
            # split into bf16 hi/lo AND permute into q-order (col q = j*128+p
            # holds S_{c(p,j)}) so the correction matmul reads contiguously
            Sh = shlpool.tile([96, NCH], BF16, tag="sh")
            Sl = shlpool.tile([96, NCH], BF16, tag="sl")
            sgath = (wb[0:96, 0:NCH]
                     .rearrange("r (p j) -> r p j", j=JG)
                     .transpose([0, 2, 1]))          # (96, j, p): S_{c(p,j)}
            shv = Sh[:, :].rearrange("r (j p) -> r j p", p=ROWS)
            slv = Sl[:, :].rearrange("r (j p) -> r j p", p=ROWS)
            nc.scalar.copy(shv, sgath)
            nc.vector.tensor_sub(out=slv, in0=sgath, in1=shv)

            # ---- phase B: YT = Phi x + Gamma S, then transpose back ----
            with tc.tile_pool(name="ytp", bufs=4, space="PSUM") as ytpsum, \
                 tc.tile_pool(name="ynp", bufs=4, space="PSUM") as ynpsum, \
                 tc.tile_pool(name="yts", bufs=3) as ytpool:
                nseq_b = 0 if ablate in ("A", "AS") else SEQ_PER_CORE
                for sq in range(nseq_b):
                    yt = ytpool.tile([ROWS, COLS], F32, tag="yt")
                    for i in range(4):
                        isl = slice(i * 512, (i + 1) * 512)
                        ytp = ytpsum.tile([128, 512], F32, tag="ytp")
                        terms = (
                            (wsl(toepT_sb, 0, sq, L), XTh[sq][:, isl]),
                            (wsl(toepT_sb, 0, sq, L), XTl[sq][:, isl]),
                            (wsl(toepT_sb, 1, sq, L), XTh[sq][:, isl]),
                            (wsl(gammaT_sb, 0, sq, L), Sh[:, isl]),
                            (wsl(gammaT_sb, 0, sq, L), Sl[:, isl]),
                            (wsl(gammaT_sb, 1, sq, L), Sh[:, isl]),
                        )
                        for k, (lw, rx) in enumerate(terms):
                            nc.tensor.matmul(ytp[:], lhsT=lw, rhs=rx,
                                             start=(k == 0),
                                             stop=(k == len(terms) - 1))
                        if i % 2 == 0:
                            nc.scalar.copy(yt[:, isl], ytp)
                        else:
                            nc.vector.tensor_copy(out=yt[:, isl], in_=ytp[:])
                    ysb = ypool.tile([ROWS, COLS], F32, tag="ysb")
                    for g in range(4):
                        ynp = ynpsum.tile([128, 512], F32, tag="ynp")
                        for jj in range(4):
                            j = 4 * g + jj
                            nc.tensor.transpose(
                                ynp[:, jj * 128:(jj + 1) * 128],
                                yt[:, j * 128:(j + 1) * 128], ident_sb)
                        if g % 2 == 0:
                            nc.scalar.copy(ysb[:, g * 512:(g + 1) * 512], ynp)
                        else:
                            nc.vector.tensor_copy(
                                out=ysb[:, g * 512:(g + 1) * 512], in_=ynp[:])
                    nc.sync.dma_start(out=y_d[sq], in_=ysb)


# ----------------------------------------------------------------- entry point
class BassRunner:
    """Builds the sharded jitted executable for a compiled Bass module once;
    subsequent calls only device_put inputs and execute."""

    def __init__(self, nc, n_cores=N_CORES):
        import jax
        from jax.experimental.shard_map import shard_map
        from jax.sharding import Mesh, PartitionSpec
        from concourse.bass2jax import (_bass_exec_p, install_neuronx_cc_hook,
                                        partition_id_tensor)
        install_neuronx_cc_hook()
        self.jax = jax
        partition_name = (nc.partition_id_tensor.name
                          if nc.partition_id_tensor else None)
        in_names, out_names, out_avals, zero_outs = [], [], [], []
        for alloc in nc.m.functions[0].allocations:
            if not isinstance(alloc, mybir.MemoryLocationSet):
                continue
            name = alloc.memorylocations[0].name
            if alloc.kind == "ExternalInput":
                if name != partition_name:
                    in_names.append(name)
            elif alloc.kind == "ExternalOutput":
                out_names.append(name)
                shape = tuple(alloc.tensor_shape)
                dtype = mybir.dt.np(alloc.dtype)
                out_avals.append(jax.core.ShapedArray(shape, dtype))
                zero_outs.append(np.zeros(shape, dtype))
        self.in_names, self.out_names = in_names, out_names
        self.out_avals, self.zero_outs = out_avals, zero_outs
        all_in_names = list(in_names) + list(out_names)
        if partition_name is not None:
            all_in_names.append(partition_name)

        def _body(*args):
            operands = list(args)
            if partition_name is not None:
                operands.append(partition_id_tensor())
            return tuple(_bass_exec_p.bind(
                *operands, out_avals=tuple(out_avals),
                in_names=tuple(all_in_names), out_names=tuple(out_names),
                lowering_input_output_aliases=(),
                sim_require_finite=True, sim_require_nnan=True, nc=nc))

        devices = jax.devices()[:n_cores]
        mesh = Mesh(np.asarray(devices), ("core",))
        nin = len(in_names) + len(out_names)
        self.fn = jax.jit(
            shard_map(_body, mesh=mesh,
                      in_specs=(PartitionSpec("core"),) * nin,
                      out_specs=(PartitionSpec("core"),) * len(out_names),
                      check_rep=False),
            keep_unused=True)
        self.n_cores = n_cores

    def concat_args(self, in_maps):
        args = [np.concatenate([np.asarray(in_maps[c][nm])
                                for c in range(self.n_cores)], axis=0)
                for nm in self.in_names]
        args += [np.zeros((self.n_cores * z.shape[0], *z.shape[1:]), z.dtype)
                 for z in self.zero_outs]
        return args

    def __call__(self, in_maps):
        outs = self.fn(*self.concat_args(in_maps))
        self.jax.block_until_ready(outs)
        return outs


_RUNNER_CACHE = {}


def _get_runner(rep=1):
    if rep not in _RUNNER_CACHE:
        _RUNNER_CACHE[rep] = BassRunner(build_nc(rep=rep))
    return _RUNNER_CACHE[rep]



def _prepare_in_maps(x, p, W, b, sample_rate):
    bc, ac = _coeffs_from_inputs(p, W, b, sample_rate)
    A, Bv, Cv, D = _state_space(bc, ac)
    h, Gam, M, Pd = _derived(A, Bv, Cv, D)
    toepT, gammaT, mT, scanP = _pack_weights(h, Gam, M, Pd)
    toepT_hl = np.stack(_split_hi_lo(toepT))      # (2, nb, 128, 128) bf16
    gammaT_hl = np.stack(_split_hi_lo(gammaT))    # (2, nb, 96, 128)
    mT_hl = np.stack(_split_hi_lo(mT))            # (2, nb, 128, 96)
    ident = np.eye(128, dtype=np.float32)
    # chunk-column layout: xt[s][m, j*128+p] = x[s, p*2048 + j*128 + m]
    # (digit-reversed chunk order c = p*16+j, matching the device views)
    x4 = x.reshape(B * C, ROWS, JG, L).astype(np.float32)
    xt = np.ascontiguousarray(x4.transpose(0, 3, 2, 1)).reshape(
        B * C, L, COLS)
    xs_h, xs_l = _split_hi_lo(xt)
    in_maps = []
    for core in range(N_CORES):
        sl = slice(core * SEQ_PER_CORE, (core + 1) * SEQ_PER_CORE)
        in_maps.append({
            "xh": np.ascontiguousarray(xs_h[sl]),
            "xl": np.ascontiguousarray(xs_l[sl]),
            "toepT": np.ascontiguousarray(toepT_hl[:, sl]),
            "gammaT": np.ascontiguousarray(gammaT_hl[:, sl]),
            "mT": np.ascontiguousarray(mT_hl[:, sl]),
            "scanP": np.ascontiguousarray(scanP[core]),
            "ident": ident,
        })
    return in_maps


def kernel(x, p, W, b, sample_rate):
    runner = _get_runner(rep=1)
    in_maps = _prepare_in_maps(x, p, W, b, sample_rate)
    outs = runner(in_maps)
    y = np.asarray(outs[0]).reshape(B * C, T)
    return y.reshape(B, C, T).astype(np.float32)



# revision 18
# speedup vs baseline: 25.8825x; 25.8825x over previous
"""Trainium2 Bass kernel for nn_AutodiffChannel: 6-biquad EQ cascade over
(64, 1, 262144) fp32 audio, data-parallel over 8 NeuronCores.

Algorithm (per sequence, LTI block-state decomposition):
  The 6-stage DF2T biquad cascade is a 12-state linear system
  s' = A s + B x, y = C s + D x.  Split T=262144 into 2048 chunks of
  L=128.  Then per chunk c:
      y_c = Phi x_c + Gamma S_c          (Phi  = 128x128 lower-tri Toeplitz
                                          of the impulse response h[0:128],
                                          Gamma[m,:] = C A^m)
      U_c = M x_c                        (M[:,n] = A^(127-n) B)
      S_c = sum_{j<c} (A^128)^(c-1-j) U_j   (exclusive prefix "state scan")
  The prefix is computed with a Kogge-Stone scan (11 levels) using
  precomputed powers P_d = (A^128)^(2^d).  The tiny per-sequence setup
  (h, Gamma, M, P_d) is computed host-side in float64.

Device dataflow per core (8 sequences), lean-precision variant:
  x arrives as bf16 (hi part only; the bf16 truncation of x contributes
  ~2e-3 relative error, inside the 2e-2 budget).  All x-side matmuls use
  2-term bf16 weight splits (Wh+Wl, ~2^-17 weight error).  U = M x
  accumulates all 8 seqs into one 96-row PSUM tile per 512-column block.
  The Kogge-Stone scan stays fp32 and is double-buffered + 512-blocked:
  each level's matmul blocks pipeline with the adds (spread over
  DVE/Pool) so the PE never waits for the adds.  S is split into bf16
  hi/lo; the output FIR y = Phi x + Gamma S uses 5 bf16 terms
  (Th x, Tl x, Gh Sh, Gl Sh, Gh Sl; ~3.6e-3 total rel err).  y is
  written back in chunk-column layout as bf16; the host does the final
  (free) transpose back to natural layout and the fp32 cast.
"""
import sys

for _p in ("/opt/trn_rl_repo", "/opt/trn_rl_repo/concourse"):
    if _p not in sys.path:
        sys.path.insert(0, _p)

import numpy as np

import concourse.bacc as bacc
import concourse.mybir as mybir
from concourse.tile import TileContext
from concourse.bass_utils import run_bass_kernel_spmd  # noqa: F401 (env check)

# ---------------------------------------------------------------- problem dims
B, C, T = 64, 1, 262144
N_CORES = 8
SEQ_PER_CORE = B * C // N_CORES  # 8
L = 128                     # chunk length
NCH = T // L                # 2048 chunks per sequence
ROWS = 128                  # partitions: within-chunk sample index
COLS = NCH                  # 2048 chunk columns
LEVELS = 11                 # ceil(log2(NCH))
NSTATE = 12
BLK = 512                   # column blocking (1 PSUM bank of fp32)
NBLK = COLS // BLK
F32 = mybir.dt.float32
BF16 = mybir.dt.bfloat16

PARAM_RANGES = np.array([
    [-24.0, 24.0], [20.0, 200.0], [0.1, 10.0],
    [-24.0, 24.0], [200.0, 2000.0], [0.1, 10.0],
    [-24.0, 24.0], [200.0, 2000.0], [0.1, 10.0],
    [-24.0, 24.0], [2000.0, 8000.0], [0.1, 10.0],
    [-24.0, 24.0], [4000.0, 12000.0], [0.1, 10.0],
    [-24.0, 24.0], [4000.0, 12000.0], [0.1, 10.0],
], dtype=np.float32)
FILTER_TYPES = ["low_shelf", "peaking", "peaking", "peaking", "peaking",
                "high_shelf"]


# ------------------------------------------------------------- host-side setup
def _sigmoid_f32(z):
    z = z.astype(np.float32)
    out = np.empty_like(z)
    pos = z >= 0
    out[pos] = (np.float32(1.0) / (np.float32(1.0) + np.exp(-z[pos]))).astype(
        np.float32)
    ez = np.exp(z[~pos]).astype(np.float32)
    out[~pos] = (ez / (np.float32(1.0) + ez)).astype(np.float32)
    return out


def _biquad_coeffs_f32(g, f, q, sr, ftype):
    """fp32-faithful audio-EQ-cookbook coefficients (matches reference)."""
    f32 = np.float32
    A = np.power(f32(10.0), (g / f32(40.0)).astype(f32)).astype(f32)
    w0 = (f32(2.0) * f32(np.pi) * (f / f32(sr))).astype(f32)
    alpha = (np.sin(w0, dtype=f32) / (f32(2.0) * q)).astype(f32)
    c = np.cos(w0, dtype=f32)
    sA = np.sqrt(A).astype(f32)
    one, two = f32(1.0), f32(2.0)
    if ftype == "low_shelf":
        b0 = A * ((A + one) - (A - one) * c + two * sA * alpha)
        b1 = two * A * ((A - one) - (A + one) * c)
        b2 = A * ((A + one) - (A - one) * c - two * sA * alpha)
        a0 = (A + one) + (A - one) * c + two * sA * alpha
        a1 = -two * ((A - one) + (A + one) * c)
        a2 = (A + one) + (A - one) * c - two * sA * alpha
    elif ftype == "high_shelf":
        b0 = A * ((A + one) + (A - one) * c + two * sA * alpha)
        b1 = -two * A * ((A - one) + (A + one) * c)
        b2 = A * ((A + one) + (A - one) * c - two * sA * alpha)
        a0 = (A + one) - (A - one) * c + two * sA * alpha
        a1 = two * ((A - one) - (A + one) * c)
        a2 = (A + one) - (A - one) * c - two * sA * alpha
    else:
        b0 = one + alpha * A
        b1 = -two * c
        b2 = one - alpha * A
        a0 = one + alpha / A
        a1 = -two * c
        a2 = one - alpha / A
    bc = (np.stack([b0, b1, b2], -1).astype(f32) / a0[..., None]).astype(f32)
    ac = (np.stack([a0, a1, a2], -1).astype(f32) / a0[..., None]).astype(f32)
    return bc, ac


def _coeffs_from_inputs(p, W, b, sample_rate):
    z = (p.astype(np.float32) @ W.astype(np.float32).T
         + b.astype(np.float32)).astype(np.float32)
    pn = _sigmoid_f32(z)
    lo, hi = PARAM_RANGES[:, 0], PARAM_RANGES[:, 1]
    params = (pn * (hi - lo) + lo).astype(np.float32)
    bcs, acs = [], []
    for k, ftype in enumerate(FILTER_TYPES):
        bc, ac = _biquad_coeffs_f32(
            params[:, 3 * k], params[:, 3 * k + 1], params[:, 3 * k + 2],
            float(sample_rate), ftype)
        bcs.append(bc)
        acs.append(ac)
    return np.stack(bcs), np.stack(acs)  # (6, B, 3) fp32


def _state_space(bc, ac):
    """Vectorized float64 (A, B, C, D) per sequence from fp32 DF2T coeffs."""
    nb = bc.shape[1]
    bc64 = bc.astype(np.float64)
    ac64 = ac.astype(np.float64)

    def step(s, x):
        s = s.copy()
        v = x
        for k in range(6):
            b0, b1, b2 = bc64[k, :, 0], bc64[k, :, 1], bc64[k, :, 2]
            a1, a2 = ac64[k, :, 1], ac64[k, :, 2]
            s1, s2 = s[:, 2 * k], s[:, 2 * k + 1]
            y = b0 * v + s1
            s[:, 2 * k] = b1 * v - a1 * y + s2
            s[:, 2 * k + 1] = b2 * v - a2 * y
            v = y
        return s, v

    A = np.zeros((nb, NSTATE, NSTATE))
    Cv = np.zeros((nb, NSTATE))
    for i in range(NSTATE):
        e = np.zeros((nb, NSTATE))
        e[:, i] = 1.0
        sp, y = step(e, np.zeros(nb))
        A[:, :, i] = sp
        Cv[:, i] = y
    Bv, D = step(np.zeros((nb, NSTATE)), np.ones(nb))
    return A, Bv, Cv, D


def _derived(A, Bv, Cv, D):
    """h (nb,L), Gamma (nb,L,12), M (nb,12,L), Pd (nb,LEVELS,12,12) in f64."""
    nb = A.shape[0]
    h = np.zeros((nb, L))
    Gam = np.zeros((nb, L, NSTATE))
    M = np.zeros((nb, NSTATE, L))
    h[:, 0] = D
    cam = Cv.copy()          # C A^m
    amb = Bv.copy()          # A^m B
    for m in range(L):
        Gam[:, m, :] = cam
        M[:, :, L - 1 - m] = amb
        if m + 1 < L:
            h[:, m + 1] = np.einsum("bi,bi->b", cam, Bv)
        cam = np.einsum("bi,bij->bj", cam, A)
        amb = np.einsum("bij,bj->bi", A, amb)
    sq = A.copy()
    for _ in range(7):       # A^(2^7) = A^128
        sq = sq @ sq
    Pd = np.zeros((nb, LEVELS, NSTATE, NSTATE))
    for d in range(LEVELS):
        Pd[:, d] = sq
        sq = sq @ sq
    return h, Gam, M, Pd


def _split_hi_lo(a):
    """Split fp32 into bf16 hi + bf16 lo (a ~= hi + lo, ~17-bit mantissa)."""
    import ml_dtypes
    a = a.astype(np.float32)
    hi = a.astype(ml_dtypes.bfloat16)
    lo = (a - hi.astype(np.float32)).astype(ml_dtypes.bfloat16)
    return hi, lo


def _pack_weights(h, Gam, M, Pd):
    """fp32 device weight tensors, per core."""
    nb = h.shape[0]
    m_idx = np.arange(L)
    diff = m_idx[None, :] - m_idx[:, None]          # [n, m] = m - n
    toepT = np.where(diff >= 0, h[:, np.clip(diff, 0, L - 1)],
                     0.0).astype(np.float32)        # (nb, n=128, m=128)
    # embedded at per-seq 12-row offsets inside a 96-row frame so every
    # device access stays at base partition 0 (HW requires 32-aligned bases)
    gammaT = np.zeros((nb, 96, L), np.float32)      # (nb, k-embed, m)
    mT = np.zeros((nb, L, 96), np.float32)          # (nb, n, k-embed)
    for g in range(nb):
        s8 = g % SEQ_PER_CORE
        gammaT[g, 12 * s8:12 * s8 + 12, :] = Gam[g].T.astype(np.float32)
        mT[g, :, 12 * s8:12 * s8 + 12] = M[g].T.astype(np.float32)
    scanP = np.zeros((N_CORES, LEVELS, 96, 96), np.float32)
    for core in range(N_CORES):
        for s in range(SEQ_PER_CORE):
            g = core * SEQ_PER_CORE + s
            for d in range(LEVELS):
                scanP[core, d, 12 * s:12 * s + 12, 12 * s:12 * s + 12] = \
                    Pd[g, d].T.astype(np.float32)
    return toepT, gammaT, mT, scanP


# ------------------------------------------------------------ device kernel IR
_NC_CACHE = {}


def build_nc(rep=1, ablate=""):
    key = (rep, ablate)
    if key in _NC_CACHE:
        return _NC_CACHE[key]
    nc = bacc.Bacc("TRN2")
    # block-major packed x/y: [blk, row, seq, col] gives every DMA 8 KiB of
    # contiguous DRAM per partition row (8 seqs x 512 cols x 2B); weights
    # arrive pre-packed in their exact SBUF layouts (contiguous DMAs)
    xh_d = nc.dram_tensor("xh", [NBLK, ROWS, SEQ_PER_CORE, BLK], BF16,
                          kind="ExternalInput")
    toepT_d = nc.dram_tensor("toepT", [L, 2 * SEQ_PER_CORE * L], BF16,
                             kind="ExternalInput")
    gammaT_d = nc.dram_tensor("gammaT", [96, 2 * SEQ_PER_CORE * L], BF16,
                              kind="ExternalInput")
    mT_d = nc.dram_tensor("mT", [L, 2 * SEQ_PER_CORE * 96], BF16,
                          kind="ExternalInput")
    scanP_d = nc.dram_tensor("scanP", [96, LEVELS * 96], F32,
                             kind="ExternalInput")
    y_d = nc.dram_tensor("y", [NBLK, ROWS, SEQ_PER_CORE, BLK], BF16,
                         kind="ExternalOutput")

    with TileContext(nc) as tc:
        with tc.tile_pool(name="weights", bufs=1) as wpool:
            toepT_sb = wpool.tile([L, 2 * SEQ_PER_CORE * L], BF16)
            gammaT_sb = wpool.tile([96, 2 * SEQ_PER_CORE * L], BF16)
            mT_sb = wpool.tile([L, 2 * SEQ_PER_CORE * 96], BF16)
            scanP_sb = wpool.tile([96, LEVELS * 96], F32)

            def dma_weights():
                # phase-A-critical weights first; FIR weights can trail
                nc.sync.dma_start(out=mT_sb, in_=mT_d[:])
                nc.sync.dma_start(out=scanP_sb, in_=scanP_d[:])
                nc.sync.dma_start(out=toepT_sb, in_=toepT_d[:])
                nc.sync.dma_start(out=gammaT_sb, in_=gammaT_d[:])

            with tc.tile_pool(name="xt", bufs=1) as xtpool, \
                 tc.tile_pool(name="ysb", bufs=2) as ypool:
                for r in range(rep):
                    _one_pass(nc, tc, xh_d, y_d, toepT_sb, gammaT_sb,
                              mT_sb, scanP_sb, xtpool, ypool, ablate,
                              dma_weights if r == 0 else None)
    nc.compile()
    _NC_CACHE[key] = nc
    return nc


def _one_pass(nc, tc, xh_d, y_d, toepT_sb, gammaT_sb, mT_sb, scanP_sb,
              xtpool, ypool, ablate="", dma_weights=None):
    # single x tile, column index = blk*4096 + sq*512 + j
    XT = xtpool.tile([ROWS, SEQ_PER_CORE * COLS], BF16, tag="xt", name="xt")
    for i in range(NBLK):
        nc.sync.dma_start(
            out=XT[:, i * 8 * BLK:(i + 1) * 8 * BLK], in_=xh_d[i])
    if dma_weights is not None:
        dma_weights()

    def xsl(sq, i):
        off = (i * SEQ_PER_CORE + sq) * BLK
        return XT[:, off:off + BLK]

    def wsl(base, h, sq, width):     # hi/lo weight slice helper
        off = (h * SEQ_PER_CORE + sq) * width
        return base[:, off:off + width]

    with tc.tile_pool(name="wbuf", bufs=1) as wbpool:
        # scan state (in place): col 0 = zeros, col 1+c = U_c then the
        # inclusive prefix W_c; rows 12s..12s+12 = seq s
        wb = wbpool.tile([96, NCH + 1], F32, tag="wb")
        # fp32 spill of the scan-independent FIR part Phi x (per seq)
        yx = wbpool.tile([ROWS, SEQ_PER_CORE * COLS], F32, tag="yx")
        nc.gpsimd.memset(wb[:, 0:1], 0.0)

        # ---- phase A: U_c = M x_c, all 8 seqs into one 96-row tile ----
        with tc.tile_pool(name="up", bufs=2, space="PSUM") as upsum:
            for i in range(NBLK):
                up = upsum.tile([96, BLK], F32, tag="up")
                k = 0
                for sq in range(SEQ_PER_CORE):
                    for h in range(2):
                        nc.tensor.matmul(
                            up[:], lhsT=wsl(mT_sb, h, sq, 96),
                            rhs=xsl(sq, i),
                            start=(k == 0), stop=(k == 2 * SEQ_PER_CORE - 1))
                        k += 1
                # Pool/GpSimd cannot read PSUM: copies must use DVE or Act
                if i % 2 == 0:
                    nc.vector.tensor_copy(
                        out=wb[:, 1 + i * BLK:1 + (i + 1) * BLK], in_=up[:])
                else:
                    nc.scalar.copy(wb[:, 1 + i * BLK:1 + (i + 1) * BLK],
                                   up[:])

        if ablate == "A":
            return

        with tc.tile_pool(name="shl", bufs=1) as shlpool, \
             tc.tile_pool(name="sp", bufs=2, space="PSUM") as spsum, \
             tc.tile_pool(name="ytp", bufs=4, space="PSUM") as ytpsum:
            Sh = shlpool.tile([96, NCH], BF16, tag="sh")
            Sl = shlpool.tile([96, NCH], BF16, tag="sl")
            # wb cols 1..2048 as a scan array W[1..N]
            wbv = wb[:, 1:NCH + 1]

            fx_jobs = [(sq, i) for i in range(NBLK)
                       for sq in range(SEQ_PER_CORE)]
            fx_done = [0]

            def emit_fx(n):
                # Phi x terms: independent of the scan; spliced into the
                # Brent-Kung serial gaps to keep the PE busy
                for _ in range(n):
                    if fx_done[0] >= len(fx_jobs):
                        return
                    sq, i = fx_jobs[fx_done[0]]
                    fx_done[0] += 1
                    ytp = ytpsum.tile([128, BLK], F32, tag="ytp")
                    for k, h in enumerate((0, 1)):
                        nc.tensor.matmul(ytp[:], lhsT=wsl(toepT_sb, h, sq, L),
                                         rhs=xsl(sq, i), start=(k == 0),
                                         stop=(k == 1))
                    if (sq + i) % 2 == 0:
                        nc.vector.tensor_copy(out=xsl_yx(sq, i), in_=ytp[:])
                    else:
                        nc.scalar.copy(xsl_yx(sq, i), ytp[:])

            def xsl_yx(sq, i):
                off = (i * SEQ_PER_CORE + sq) * BLK
                return yx[:, off:off + BLK]

            def bk_piece(d, k0, nk, down):
                stride = 2 << d
                v = wbv.rearrange("p (k s) -> p k s", s=stride)
                if down:
                    # W[2^{d+1}k + 2^d] += P_d W[2^{d+1}k], k >= 1
                    tgt = v[:, k0 + 1:k0 + 1 + nk,
                            (stride >> 1) - 1:(stride >> 1)]
                    src = v[:, k0:k0 + nk, stride - 1:stride]
                else:
                    # W[2^{d+1}(k+1)] += P_d W[2^{d+1}(k+1) - 2^d]
                    tgt = v[:, k0:k0 + nk, stride - 1:stride]
                    src = v[:, k0:k0 + nk, (stride >> 1) - 1:(stride >> 1)]
                sp = spsum.tile([96, BLK], F32, tag="sp")
                nc.tensor.matmul(sp[:, 0:nk],
                                 lhsT=scanP_sb[:, d * 96:(d + 1) * 96],
                                 rhs=src, start=True, stop=True)
                nc.vector.tensor_add(
                    out=tgt, in0=tgt,
                    in1=sp[:, 0:nk].rearrange("p (n o) -> p n o", o=1))

            def emit_split(blk):
                csl = slice(blk * BLK, (blk + 1) * BLK)
                nc.scalar.copy(Sh[:, csl], wb[:, csl])
                # SBUF-only op: Pool, keeping DVE free
                nc.gpsimd.tensor_sub(out=Sl[:, csl], in0=wb[:, csl],
                                     in1=Sh[:, csl])

            with tc.tile_pool(name="gtmp", bufs=2) as gtpool:

                def emit_gamma(i):
                    isl = slice(i * BLK, (i + 1) * BLK)
                    ysb = ypool.tile([ROWS, SEQ_PER_CORE * BLK], BF16,
                                     tag="ysb")
                    for sq in range(SEQ_PER_CORE):
                        ytp = ytpsum.tile([128, BLK], F32, tag="ytp")
                        terms = ((0, Sh[:, isl]), (1, Sh[:, isl]),
                                 (0, Sl[:, isl]))
                        for k, (h, rx) in enumerate(terms):
                            nc.tensor.matmul(ytp[:],
                                             lhsT=wsl(gammaT_sb, h, sq, L),
                                             rhs=rx, start=(k == 0),
                                             stop=(k == len(terms) - 1))
                        yband = ysb[:, sq * BLK:(sq + 1) * BLK]
                        if (i * SEQ_PER_CORE + sq) % 2 == 0:
                            nc.vector.tensor_add(out=yband, in0=ytp[:],
                                                 in1=xsl_yx(sq, i))
                        else:
                            # spread the PSUM drain over Act + Pool
                            gt = gtpool.tile([128, BLK], F32, tag="gt")
                            nc.scalar.copy(gt[:], ytp[:])
                            nc.gpsimd.tensor_add(out=yband, in0=gt[:],
                                                 in1=xsl_yx(sq, i))
                    nc.sync.dma_start(out=y_d[i], in_=ysb[:])

                # ---- Brent-Kung up-sweep ----
                for d in range(LEVELS):
                    n, k0 = NCH // (2 << d), 0
                    while n > 0:
                        nk = min(n, BLK)
                        bk_piece(d, k0, nk, down=False)
                        k0 += nk
                        n -= nk
                    emit_fx(1 if d > 0 else 0)

                # ---- down-sweep (d = 0 split for early Gamma start) ----
                for d in range(LEVELS - 2, 0, -1):
                    m, k0 = NCH // (2 << d) - 1, 0
                    while m > 0:
                        nk = min(m, BLK)
                        bk_piece(d, k0, nk, down=True)
                        k0 += nk
                        m -= nk
                    emit_fx(3)
                bk_piece(0, 0, BLK, down=True)       # finalizes W[1..1025]
                emit_split(0)
                emit_split(1)
                bk_piece(0, BLK, NCH // 2 - 1 - BLK, down=True)
                emit_fx(len(fx_jobs))   # flush remaining Phi x work
                if ablate == "AS":
                    emit_split(2)
                    emit_split(3)
                    return
                emit_gamma(0)
                emit_split(2)
                emit_split(3)
                emit_gamma(1)
                emit_gamma(2)
                emit_gamma(3)


# ----------------------------------------------------------------- entry point
class BassRunner:
    """Builds the sharded jitted executable for a compiled Bass module once;
    subsequent calls only device_put inputs and execute."""

    def __init__(self, nc, n_cores=N_CORES):
        import jax
        from jax.experimental.shard_map import shard_map
        from jax.sharding import Mesh, PartitionSpec
        from concourse.bass2jax import (_bass_exec_p, install_neuronx_cc_hook,
                                        partition_id_tensor)
        install_neuronx_cc_hook()
        self.jax = jax
        partition_name = (nc.partition_id_tensor.name
                          if nc.partition_id_tensor else None)
        in_names, out_names, out_avals, zero_outs = [], [], [], []
        for alloc in nc.m.functions[0].allocations:
            if not isinstance(alloc, mybir.MemoryLocationSet):
                continue
            name = alloc.memorylocations[0].name
            if alloc.kind == "ExternalInput":
                if name != partition_name:
                    in_names.append(name)
            elif alloc.kind == "ExternalOutput":
                out_names.append(name)
                shape = tuple(alloc.tensor_shape)
                dtype = mybir.dt.np(alloc.dtype)
                out_avals.append(jax.core.ShapedArray(shape, dtype))
                zero_outs.append(np.zeros(shape, dtype))
        self.in_names, self.out_names = in_names, out_names
        self.out_avals, self.zero_outs = out_avals, zero_outs
        all_in_names = list(in_names) + list(out_names)
        if partition_name is not None:
            all_in_names.append(partition_name)

        def _body(*args):
            operands = list(args)
            if partition_name is not None:
                operands.append(partition_id_tensor())
            return tuple(_bass_exec_p.bind(
                *operands, out_avals=tuple(out_avals),
                in_names=tuple(all_in_names), out_names=tuple(out_names),
                lowering_input_output_aliases=(),
                sim_require_finite=True, sim_require_nnan=True, nc=nc))

        devices = jax.devices()[:n_cores]
        mesh = Mesh(np.asarray(devices), ("core",))
        nin = len(in_names) + len(out_names)
        self.fn = jax.jit(
            shard_map(_body, mesh=mesh,
                      in_specs=(PartitionSpec("core"),) * nin,
                      out_specs=(PartitionSpec("core"),) * len(out_names),
                      check_rep=False),
            keep_unused=True)
        self.n_cores = n_cores

    def concat_args(self, in_maps):
        args = [np.concatenate([np.asarray(in_maps[c][nm])
                                for c in range(self.n_cores)], axis=0)
                for nm in self.in_names]
        args += [np.zeros((self.n_cores * z.shape[0], *z.shape[1:]), z.dtype)
                 for z in self.zero_outs]
        return args

    def __call__(self, in_maps):
        outs = self.fn(*self.concat_args(in_maps))
        self.jax.block_until_ready(outs)
        return outs


_RUNNER_CACHE = {}


def _get_runner(rep=1):
    if rep not in _RUNNER_CACHE:
        _RUNNER_CACHE[rep] = BassRunner(build_nc(rep=rep))
    return _RUNNER_CACHE[rep]


def _prepare_in_maps(x, p, W, b, sample_rate):
    import ml_dtypes
    bc, ac = _coeffs_from_inputs(p, W, b, sample_rate)
    A, Bv, Cv, D = _state_space(bc, ac)
    h, Gam, M, Pd = _derived(A, Bv, Cv, D)
    toepT, gammaT, mT, scanP = _pack_weights(h, Gam, M, Pd)
    toepT_hl = np.stack(_split_hi_lo(toepT))      # (2, nb, 128, 128) bf16
    gammaT_hl = np.stack(_split_hi_lo(gammaT))    # (2, nb, 96, 128)
    mT_hl = np.stack(_split_hi_lo(mT))            # (2, nb, 128, 96)
    # chunk-column layout xt[s][m, c] = x[s, c*128 + m], then packed
    # block-major [blk, row, seq, col] per core for contiguous 8 KiB DMAs
    x4 = x.reshape(B * C, NCH, L).astype(np.float32)
    xt = x4.transpose(0, 2, 1)                        # (nb, 128, 2048)
    xs_h = xt.astype(ml_dtypes.bfloat16)
    in_maps = []
    for core in range(N_CORES):
        sl = slice(core * SEQ_PER_CORE, (core + 1) * SEQ_PER_CORE)
        xpk = np.ascontiguousarray(
            xs_h[sl].reshape(SEQ_PER_CORE, ROWS, NBLK, BLK)
            .transpose(2, 1, 0, 3))                   # (blk, row, seq, col)
        # weights pre-packed into their exact SBUF layouts: row = partition,
        # col = (h, s, m) resp. (d, k) -> plain contiguous DMAs on device
        in_maps.append({
            "xh": xpk,
            "toepT": np.ascontiguousarray(
                toepT_hl[:, sl].transpose(2, 0, 1, 3).reshape(L, -1)),
            "gammaT": np.ascontiguousarray(
                gammaT_hl[:, sl].transpose(2, 0, 1, 3).reshape(96, -1)),
            "mT": np.ascontiguousarray(
                mT_hl[:, sl].transpose(2, 0, 1, 3).reshape(L, -1)),
            "scanP": np.ascontiguousarray(
                scanP[core].transpose(1, 0, 2).reshape(96, -1)),
        })
    return in_maps


def unpack_y(ypk_all):
    """(n_cores*NBLK, ROWS, SEQ, BLK) packed bf16 -> (B, C, T) fp32."""
    ypk = np.asarray(ypk_all).reshape(N_CORES, NBLK, ROWS, SEQ_PER_CORE, BLK)
    # yq[s][m, blk*512+j] = ypk[blk, m, s, j];  y[s, c*128+m] = yq[s][m, c]
    yq = ypk.transpose(0, 3, 2, 1, 4).reshape(B * C, ROWS, NCH)
    y = np.ascontiguousarray(yq.transpose(0, 2, 1)).astype(np.float32)
    return y.reshape(B, C, T)


def kernel(x, p, W, b, sample_rate):
    runner = _get_runner(rep=1)
    in_maps = _prepare_in_maps(x, p, W, b, sample_rate)
    outs = runner(in_maps)
    return unpack_y(outs[0])
